# revision 43
# baseline (speedup 1.0000x reference)
"""Multi-head attention Trainium2 kernel (B=2, S=2048, D=1024, H=16, Dh=64).

Sharding: 8 cores = 2 (batch) x 4 (head-groups of 4 heads).
Each core computes qT/kT/v projections for its 4 heads, attention
(scoresT layout, fused softmax-sum via an extra ones-column in V),
and a partial (row-sharded) output projection. Host sums the 4 head-group
partials per batch and adds bo.

Schedule: fine-grained software pipeline. Attention for pair 0 / block 0
starts after a ~24-matmul prologue; the remaining k/v projections stream
just-in-time inside block 0, and q/output projections are interleaved as
PE filler between score/AV groups so the tensor engine never stalls on
the Activation engine's exp. Softmax normalization is staged through
SBUF with DMA partition shifts and per-block batched reciprocals; the
output-projection PSUM drains through the (otherwise idle) Pool engine.
"""

import sys

sys.path.insert(0, "/opt/trn_rl_repo")

import ml_dtypes
import numpy as np

import concourse.bass as bass  # noqa: F401
import concourse.mybir as mybir
import concourse.tile as tile
from concourse import bacc, bass_utils

F32 = mybir.dt.float32
BF16 = mybir.dt.bfloat16
AF = mybir.ActivationFunctionType

B, S, D = 2, 2048, 1024
H, DH = 16, 64
N_CORES = 8
HPC = 4  # heads per core
CW = HPC * DH  # c-width per core (256)
SBLK = 512  # s_q block size
NSBLK = S // SBLK  # 4
NKT = S // 128  # 16 s_k tiles
KD = D // 128  # 8 contraction tiles for projections
SC_G = 2  # s_k tiles per scores psum group
NGRP = NKT // SC_G  # 8 groups per (pair, block)

_CACHE = {}


def _build_program():
    nc = bacc.Bacc("TRN2", target_bir_lowering=False, debug=False, num_devices=N_CORES)

    qT_d = nc.dram_tensor("qT", [D, S], BF16, kind="ExternalInput").ap()
    kTx_d = nc.dram_tensor("kTx", [D, S], BF16, kind="ExternalInput").ap()
    vTx_d = nc.dram_tensor("vTx", [D, S], BF16, kind="ExternalInput").ap()
    wq_d = nc.dram_tensor("wqT", [D, CW], BF16, kind="ExternalInput").ap()
    wk_d = nc.dram_tensor("wkT", [D, CW], BF16, kind="ExternalInput").ap()
    wv_d = nc.dram_tensor("wvT", [D, CW], BF16, kind="ExternalInput").ap()
    wo_d = nc.dram_tensor("woT", [CW, D], BF16, kind="ExternalInput").ap()
    bq_d = nc.dram_tensor("bq", [CW], F32, kind="ExternalInput").ap()
    bk_d = nc.dram_tensor("bk", [CW], F32, kind="ExternalInput").ap()
    bv_d = nc.dram_tensor("bv", [CW], F32, kind="ExternalInput").ap()
    out_d = nc.dram_tensor("outT", [D, S], F32, kind="ExternalOutput").ap()

    with tile.TileContext(nc) as tc:
        _kernel_body(nc, tc, qT_d, kTx_d, vTx_d, wq_d, wk_d, wv_d, wo_d,
                     bq_d, bk_d, bv_d, out_d)
    nc.compile()
    return nc


def _kernel_body(nc, tc, qT_d, kTx_d, vTx_d, wq_d, wk_d, wv_d, wo_d,
                 bq_d, bk_d, bv_d, out_d):
    from contextlib import ExitStack

    SCALE = float(1.0 / np.sqrt(DH))
    # Schraudolph-style bf16 exp on the DVE: exp(s*SCALE) ~= bitcast_bf16(
    # int16(round(s * SCALE * 2^7/ln2 + (127*2^7 - 4.7 - 2.8)))).
    # The +1.5% mean bias of the approximation cancels in softmax since the
    # row-sums are computed from the same approximated probabilities.
    EXP_A = SCALE * 184.6650085
    EXP_B = 16248.5

    ctx = ExitStack()
    with ctx:
        const = ctx.enter_context(tc.tile_pool(name="const", bufs=1))
        persist = ctx.enter_context(tc.tile_pool(name="persist", bufs=1))
        xio = ctx.enter_context(tc.tile_pool(name="xio", bufs=3))
        vxp = ctx.enter_context(tc.tile_pool(name="vxp", bufs=4))
        strip_pool = ctx.enter_context(tc.tile_pool(name="strip", bufs=4))
        rawp = ctx.enter_context(tc.tile_pool(name="rawp", bufs=2))
        sm = ctx.enter_context(tc.tile_pool(name="sm", bufs=2))
        scps = ctx.enter_context(tc.tile_pool(name="scps", bufs=2, space="PSUM"))
        avps = ctx.enter_context(tc.tile_pool(name="avps", bufs=1, space="PSUM"))
        pjps = ctx.enter_context(tc.tile_pool(name="pjps", bufs=2, space="PSUM"))

        # ---- weights / biases ----
        wq_sb = const.tile([128, KD, CW], BF16, tag="wq")
        wk_sb = const.tile([128, KD, CW], BF16, tag="wk")
        wv_sb = const.tile([128, KD, CW], BF16, tag="wv")
        wo_sb = const.tile([128, CW // 128, D], BF16, tag="wo")
        # strict need-order: proj_k(0) first, then proj_q(0,0), then proj_v
        nc.sync.dma_start(out=wk_sb, in_=wk_d.rearrange("(k p) c -> p k c", p=128))

        # ---- persistent activations ----
        qT_sb = [persist.tile([128, S], BF16, tag=f"qT{p}", name=f"qT_sb{p}") for p in range(2)]
        kT_sb = [persist.tile([128, S], BF16, tag=f"kT{p}", name=f"kT_sb{p}") for p in range(2)]
        v_sb = persist.tile([128, NKT, HPC, DH + 1], BF16, tag="v")
        nc.vector.memset(v_sb[:, :, :, DH : DH + 1], 1.0)
        ao_sb = persist.tile([128, CW // 128, S], BF16, tag="ao")
        osb = persist.tile([128, KD, SBLK], F32, tag="osb")
        # softmax row-sums, one [2, SBLK] slot per (pair, blk)
        sums = [persist.tile([2, SBLK], F32, tag=f"sums{p}{b}", name=f"sums{p}{b}")
                for p in range(2) for b in range(NSBLK)]

        qTr = qT_d.rearrange("(k p) s -> p k s", p=128)
        kTr = kTx_d.rearrange("(k p) s -> p k s", p=128)
        vTr = vTx_d.rearrange("(k p) s -> p k s", p=128)

        # resident vx quarters (4 k-tiles each), loaded in need order
        vx_q = [vxp.tile([128, KD, S // 4], BF16, tag="vx", name=f"vxq{q}")
                for q in range(4)]

        def load_vx(q):
            nc.sync.dma_start(out=vx_q[q], in_=vTr[:, :, q * (S // 4) : (q + 1) * (S // 4)])

        xk0 = xio.tile([128, KD, SBLK], BF16, tag="x", name="xk0pre")
        nc.sync.dma_start(out=xk0, in_=kTr[:, :, 0:SBLK])
        bk_sb = const.tile([128, 2], F32, tag="bk")
        bq_sb = const.tile([128, 2], F32, tag="bq")
        for p in range(2):
            nc.sync.dma_start(out=bk_sb[:, p : p + 1],
                              in_=bk_d[p * 128 : (p + 1) * 128].unsqueeze(1))
        nc.sync.dma_start(out=wq_sb, in_=wq_d.rearrange("(k p) c -> p k c", p=128))
        xq00 = xio.tile([128, KD, SBLK], BF16, tag="x", name="xq00pre")
        nc.sync.dma_start(out=xq00, in_=qTr[:, :, 0:SBLK])
        for p in range(2):
            nc.sync.dma_start(out=bq_sb[:, p : p + 1],
                              in_=bq_d[p * 128 : (p + 1) * 128].unsqueeze(1))
        nc.sync.dma_start(out=wv_sb, in_=wv_d.rearrange("(k p) c -> p k c", p=128))
        load_vx(0)
        bv_row = const.tile([1, CW], F32, tag="bvr")
        nc.sync.dma_start(out=bv_row, in_=bv_d.unsqueeze(0))
        bv_bc = const.tile([128, CW], F32, tag="bvb")
        nc.gpsimd.partition_broadcast(bv_bc, bv_row)
        load_vx(1)
        nc.sync.dma_start(out=wo_sb, in_=wo_d.rearrange("(ct p) e -> p ct e", p=128))
        load_vx(2)
        load_vx(3)
        pre_x = {("k", 0, 0): xk0, ("k", 1, 0): xk0, ("q", 0, 0): xq00}

        # ---------------- projection helpers ----------------
        proj_state = {}

        NCHUNK = 4  # contraction chunks per q/k projection (2 matmuls each)

        def proj_qk(which, p, sb, chunk):
            """Quarter of a q/k projection (2 of 8 contraction tiles) so
            filler pops stay fine-grained (~0.5us each). chunk 0 loads x and
            starts the psum; the last chunk moves psum->SBUF with bias add."""
            w_sb, b_sb, dst, xr = (
                (wq_sb, bq_sb, qT_sb, qTr) if which == "q" else (wk_sb, bk_sb, kT_sb, kTr))
            key = (which, p, sb)
            if chunk == 0:
                x = pre_x.get(key)
                if x is None:
                    x = xio.tile([128, KD, SBLK], BF16, tag="x", name=f"x{which}{p}{sb}")
                    nc.sync.dma_start(out=x, in_=xr[:, :, sb * SBLK : (sb + 1) * SBLK])
                ps = pjps.tile([128, SBLK], F32, tag="pj", name=f"ps{which}{p}{sb}")
                proj_state[key] = (x, ps)
            x, ps = proj_state[key]
            for k in range(chunk * KD // NCHUNK, (chunk + 1) * KD // NCHUNK):
                nc.tensor.matmul(ps, w_sb[:, k, p * 128 : (p + 1) * 128],
                                 x[:, k, :], start=(k == 0), stop=(k == KD - 1))
            if chunk == NCHUNK - 1:
                nc.vector.tensor_scalar_add(dst[p][:, sb * SBLK : (sb + 1) * SBLK],
                                            ps, b_sb[:, p : p + 1])
                del proj_state[key]

        def proj_k(sb, pairs=(0, 1)):
            for p in pairs:
                for c in range(NCHUNK):
                    proj_qk("k", p, sb, c)

        def proj_q(p, sb):
            for c in range(NCHUNK):
                proj_qk("q", p, sb, c)

        def proj_v_tile(t):
            """v (natural layout) for s_k tile t, all 4 heads at once."""
            q, ti = divmod(t, NKT // 4)
            vx = vx_q[q]
            psv = pjps.tile([128, CW], F32, tag="pj", name=f"psv{t}")
            for k in range(KD):
                nc.tensor.matmul(psv, vx[:, k, ti * 128 : (ti + 1) * 128],
                                 wv_sb[:, k, :], start=(k == 0), stop=(k == KD - 1))
            nc.vector.tensor_add(
                v_sb[:, t, :, 0:DH],
                psv.rearrange("p (h c) -> p h c", h=HPC),
                bv_bc.rearrange("p (h c) -> p h c", h=HPC),
            )

        # ---------------- attention ----------------
        def attn(p, blk, filler):
            """Attention for (pair, query-block). `filler` is a list of
            zero-arg closures; a few are popped between groups to keep the
            PE busy while the Activation engine catches up on exp."""
            sq = slice(blk * SBLK, (blk + 1) * SBLK)
            avA = avps.tile([DH + 1, SBLK], F32, tag="avA", name=f"avA{p}{blk}")
            avB = avps.tile([DH + 1, SBLK], F32, tag="avB", name=f"avB{p}{blk}")
            strips = {}
            def emit_av(g):
                # AV matmuls for group g (lagged one group behind scores so
                # the PE never blocks on the Activation engine's exp)
                half = (g * SC_G) // (NKT // 2)
                expA, expB = strips[(0, half)], strips[(1, half)]
                for i in range(SC_G):
                    t = g * SC_G + i
                    th = t % (NKT // 2)
                    nc.tensor.matmul(avA, v_sb[:, t, 2 * p, :], expA[:, th, :],
                                     start=(t == 0), stop=(t == NKT - 1))
                for i in range(SC_G):
                    t = g * SC_G + i
                    th = t % (NKT // 2)
                    nc.tensor.matmul(avB, v_sb[:, t, 2 * p + 1, :], expB[:, th, :],
                                     start=(t == 0), stop=(t == NKT - 1))

            for g in range(NGRP):
                t0 = g * SC_G
                half, gh = divmod(g, NGRP // 2)
                if gh == 0:
                    strips[(0, half)] = strip_pool.tile(
                        [128, NKT // 2, SBLK], BF16, tag="exp", name=f"expA{p}{blk}{half}")
                    strips[(1, half)] = strip_pool.tile(
                        [128, NKT // 2, SBLK], BF16, tag="exp", name=f"expB{p}{blk}{half}")
                expA, expB = strips[(0, half)], strips[(1, half)]
                th0 = gh * SC_G
                scA = scps.tile([128, SC_G, SBLK], F32, tag="sc", name=f"scA{p}{blk}{g}")
                for i in range(SC_G):
                    t = t0 + i
                    nc.tensor.matmul(scA[:, i, :],
                                     kT_sb[p][0:64, t * 128 : (t + 1) * 128],
                                     qT_sb[p][0:64, sq], start=True, stop=True)
                nc.scalar.activation(expA[:, th0 : th0 + SC_G, :], scA, AF.Exp, scale=SCALE)
                scB = scps.tile([128, SC_G, SBLK], F32, tag="sc", name=f"scB{p}{blk}{g}")
                for i in range(SC_G):
                    t = t0 + i
                    nc.tensor.matmul(scB[:, i, :],
                                     kT_sb[p][64:128, t * 128 : (t + 1) * 128],
                                     qT_sb[p][64:128, sq], start=True, stop=True)
                nc.scalar.activation(expB[:, th0 : th0 + SC_G, :], scB, AF.Exp, scale=SCALE)
                # PE filler while the Activation engine computes this group's exp
                for fn in filler.pop_for_group(p, blk, g):
                    fn()
                if g > 0:
                    emit_av(g - 1)
            emit_av(NGRP - 1)
            return avA, avB

        # per (pair, blk): drain av psum to SBUF, extract row sums
        raw_tiles = {}

        def drain_av(p, blk, avA, avB):
            rawA = rawp.tile([DH + 1, SBLK], F32, tag="rawA", name=f"rawA{p}{blk}")
            rawB = rawp.tile([DH + 1, SBLK], F32, tag="rawB", name=f"rawB{p}{blk}")
            nc.vector.tensor_copy(rawA, avA)
            nc.vector.tensor_copy(rawB, avB)
            s = sums[p * NSBLK + blk]
            nc.sync.dma_start(out=s[0:1, :], in_=rawA[DH : DH + 1, :])
            nc.sync.dma_start(out=s[1:2, :], in_=rawB[DH : DH + 1, :])
            nc.vector.reciprocal_approx_fast(out=s, in_=s)
            raw_tiles[(p, blk)] = (rawA, rawB)

        def normalize(p, blk):
            sq = slice(blk * SBLK, (blk + 1) * SBLK)
            rawA, rawB = raw_tiles.pop((p, blk))
            s = sums[p * NSBLK + blk]
            for j, raw in enumerate((rawA, rawB)):
                rcp0 = sm.tile([1, SBLK], F32, tag="rcp0", name=f"rcp0{p}{blk}{j}")
                nc.sync.dma_start(out=rcp0, in_=s[j : j + 1, :])
                rcpb = sm.tile([64, SBLK], F32, tag="rcpb", name=f"rcpb{p}{blk}{j}")
                nc.gpsimd.partition_broadcast(rcpb, rcp0)
                if j == 0:
                    nc.vector.tensor_mul(ao_sb[0:64, p, sq], raw[0:DH, :], rcpb)
                else:
                    stage = sm.tile([64, SBLK], BF16, tag="stage", name=f"stage{p}{blk}{j}")
                    nc.vector.tensor_mul(stage, raw[0:DH, :], rcpb)
                    nc.sync.dma_start(out=ao_sb[64:128, p, sq], in_=stage)

        def outproj_et(blk, et, dma_now=False):
            sq = slice(blk * SBLK, (blk + 1) * SBLK)
            pso = pjps.tile([128, SBLK], F32, tag="pj", name=f"pso{blk}{et}")
            for ct in range(CW // 128):
                nc.tensor.matmul(pso, wo_sb[:, ct, et * 128 : (et + 1) * 128],
                                 ao_sb[:, ct, sq], start=(ct == 0),
                                 stop=(ct == CW // 128 - 1))
            if dma_now and et % 2 == 1:
                nc.scalar.copy(osb[:, et, :], pso)
            else:
                nc.vector.tensor_copy(osb[:, et, :], pso)
            if dma_now:
                nc.sync.dma_start(
                    out=out_d.rearrange("(et p) s -> p et s", p=128)[:, et, sq],
                    in_=osb[:, et, :])

        def out_dma(blk):
            sq = slice(blk * SBLK, (blk + 1) * SBLK)
            nc.sync.dma_start(
                out=out_d.rearrange("(et p) s -> p et s", p=128)[:, :, sq],
                in_=osb)

        # ---------------- filler scheduling ----------------
        class Filler:
            """Pinned (JIT) emissions + a generic weighted queue popped
            between groups (weight ~ matmul count, budget ~3 per group)."""

            def __init__(self):
                self.queue = []  # list of (weight, fn)

            def add(self, weight, fn):
                self.queue.append((weight, fn))

            def pop_for_group(self, p, blk, g):
                out = []
                if p == 0 and blk == 0:
                    # JIT: k-projection for upcoming s-blocks, v tiles for
                    # this group, q-projection for the next block
                    if g % 2 == 1 and g < NGRP - 1:
                        out.append(lambda sb=(g + 1) // 2: proj_k(sb))
                    if g == NGRP - 2:
                        out.append(lambda: proj_q(0, 1))
                    for t in range(g * SC_G, (g + 1) * SC_G):
                        out.append(lambda t=t: proj_v_tile(t))
                else:
                    w = 0
                    while self.queue and w < 3:
                        wi, fn = self.queue.pop(0)
                        w += wi
                        out.append(fn)
                return out

        filler = Filler()

        # ---------------- emission schedule ----------------
        proj_k(0)
        proj_q(0, 0)

        # pair 0: block 0 streams k/v projections just-in-time via Filler
        avA, avB = attn(0, 0, filler)
        drain_av(0, 0, avA, avB)

        # generic filler for pair-0 blocks 1..3: remaining q projections in
        # half-projection chunks (proj_q(0, 1) was pinned inside block 0)
        for p, sb in [(0, 2), (0, 3), (1, 0), (1, 1), (1, 2), (1, 3)]:
            filler.add(8, lambda p=p, sb=sb: proj_q(p, sb))

        for blk in range(1, NSBLK):
            avA, avB = attn(0, blk, filler)
            drain_av(0, blk, avA, avB)
            normalize(0, blk - 1)

        # pair 1 blocks; outproj(blk-1) interleaves as PE filler
        normalize(0, NSBLK - 1)
        for blk in range(NSBLK):
            if blk > 0:
                for et in range(KD):
                    filler.add(2, lambda b=blk - 1, et=et: outproj_et(b, et))
                filler.add(0, lambda b=blk - 1: out_dma(b))
            avA, avB = attn(1, blk, filler)
            drain_av(1, blk, avA, avB)
            normalize(1, blk)
        # drain remaining filler (if any), then the final block's outproj
        for _, fn in filler.queue:
            fn()
        filler.queue = []
        for et in range(KD):
            outproj_et(NSBLK - 1, et, dma_now=True)


def _get_program():
    if "nc" not in _CACHE:
        _CACHE["nc"] = _build_program()
    return _CACHE["nc"]


LAST_RESULTS = None


def kernel(query, key_, value, Wq, bq, Wk, bk, Wv, bv, Wo, bo):
    global LAST_RESULTS
    query = np.asarray(query, dtype=np.float32)
    key_ = np.asarray(key_, dtype=np.float32)
    value = np.asarray(value, dtype=np.float32)
    Wq = np.asarray(Wq, dtype=np.float32)
    Wk = np.asarray(Wk, dtype=np.float32)
    Wv = np.asarray(Wv, dtype=np.float32)
    Wo = np.asarray(Wo, dtype=np.float32)
    bq = np.asarray(bq, dtype=np.float32)
    bk = np.asarray(bk, dtype=np.float32)
    bv = np.asarray(bv, dtype=np.float32)
    bo = np.asarray(bo, dtype=np.float32)

    nc = _get_program()

    BF = ml_dtypes.bfloat16
    qT = [np.ascontiguousarray(query[b].T.astype(BF)) for b in range(B)]
    kT = [np.ascontiguousarray(key_[b].T.astype(BF)) for b in range(B)]
    vT = [np.ascontiguousarray(value[b].T.astype(BF)) for b in range(B)]

    in_maps = []
    for c in range(N_CORES):
        b, hp = divmod(c, HPC)
        cs = slice(hp * CW, (hp + 1) * CW)
        in_maps.append({
            "qT": qT[b], "kTx": kT[b], "vTx": vT[b],
            "wqT": np.ascontiguousarray(Wq[cs, :].T.astype(BF)),
            "wkT": np.ascontiguousarray(Wk[cs, :].T.astype(BF)),
            "wvT": np.ascontiguousarray(Wv[cs, :].T.astype(BF)),
            "woT": np.ascontiguousarray(Wo[:, cs].T.astype(BF)),
            "bq": np.ascontiguousarray(bq[cs]),
            "bk": np.ascontiguousarray(bk[cs]),
            "bv": np.ascontiguousarray(bv[cs]),
        })

    res = bass_utils.run_bass_kernel_spmd(nc, in_maps, core_ids=list(range(N_CORES)))
    LAST_RESULTS = res

    out = np.zeros((B, S, D), dtype=np.float32)
    for c in range(N_CORES):
        b = c // HPC
        out[b] += res.results[c]["outT"].T
    out += bo
    return out


# revision 45
# speedup vs baseline: 1.0003x; 1.0003x over previous
"""Multi-head attention Trainium2 kernel (B=2, S=2048, D=1024, H=16, Dh=64).

Sharding: 8 cores = 2 (batch) x 4 (head-groups of 4 heads).
Each core computes qT/kT/v projections for its 4 heads, attention
(scoresT layout, fused softmax-sum via an extra ones-column in V),
and a partial (row-sharded) output projection. Host sums the 4 head-group
partials per batch and adds bo.

Schedule: fine-grained software pipeline. Attention for pair 0 / block 0
starts after a ~24-matmul prologue; the remaining k/v projections stream
just-in-time inside block 0, and q/output projections are interleaved as
PE filler between score/AV groups so the tensor engine never stalls on
the Activation engine's exp. Softmax normalization is staged through
SBUF with DMA partition shifts and per-block batched reciprocals; the
output-projection PSUM drains through the (otherwise idle) Pool engine.
"""

import sys

sys.path.insert(0, "/opt/trn_rl_repo")

import ml_dtypes
import numpy as np

import concourse.bass as bass  # noqa: F401
import concourse.mybir as mybir
import concourse.tile as tile
from concourse import bacc, bass_utils

F32 = mybir.dt.float32
BF16 = mybir.dt.bfloat16
AF = mybir.ActivationFunctionType

B, S, D = 2, 2048, 1024
H, DH = 16, 64
N_CORES = 8
HPC = 4  # heads per core
CW = HPC * DH  # c-width per core (256)
SBLK = 512  # s_q block size
NSBLK = S // SBLK  # 4
NKT = S // 128  # 16 s_k tiles
KD = D // 128  # 8 contraction tiles for projections
SC_G = 2  # s_k tiles per scores psum group
NGRP = NKT // SC_G  # 8 groups per (pair, block)

_CACHE = {}


def _build_program():
    nc = bacc.Bacc("TRN2", target_bir_lowering=False, debug=False, num_devices=N_CORES)

    qT_d = nc.dram_tensor("qT", [D, S], BF16, kind="ExternalInput").ap()
    kTx_d = nc.dram_tensor("kTx", [D, S], BF16, kind="ExternalInput").ap()
    vTx_d = nc.dram_tensor("vTx", [D, S], BF16, kind="ExternalInput").ap()
    wq_d = nc.dram_tensor("wqT", [D, CW], BF16, kind="ExternalInput").ap()
    wk_d = nc.dram_tensor("wkT", [D, CW], BF16, kind="ExternalInput").ap()
    wv_d = nc.dram_tensor("wvT", [D, CW], BF16, kind="ExternalInput").ap()
    wo_d = nc.dram_tensor("woT", [CW, D], BF16, kind="ExternalInput").ap()
    bq_d = nc.dram_tensor("bq", [CW], F32, kind="ExternalInput").ap()
    bk_d = nc.dram_tensor("bk", [CW], F32, kind="ExternalInput").ap()
    bv_d = nc.dram_tensor("bv", [CW], F32, kind="ExternalInput").ap()
    out_d = nc.dram_tensor("outT", [D, S], F32, kind="ExternalOutput").ap()

    with tile.TileContext(nc) as tc:
        _kernel_body(nc, tc, qT_d, kTx_d, vTx_d, wq_d, wk_d, wv_d, wo_d,
                     bq_d, bk_d, bv_d, out_d)
    nc.compile()
    return nc


def _kernel_body(nc, tc, qT_d, kTx_d, vTx_d, wq_d, wk_d, wv_d, wo_d,
                 bq_d, bk_d, bv_d, out_d):
    from contextlib import ExitStack

    SCALE = float(1.0 / np.sqrt(DH))
    # Schraudolph-style bf16 exp on the DVE: exp(s*SCALE) ~= bitcast_bf16(
    # int16(round(s * SCALE * 2^7/ln2 + (127*2^7 - 4.7 - 2.8)))).
    # The +1.5% mean bias of the approximation cancels in softmax since the
    # row-sums are computed from the same approximated probabilities.
    EXP_A = SCALE * 184.6650085
    EXP_B = 16248.5

    ctx = ExitStack()
    with ctx:
        const = ctx.enter_context(tc.tile_pool(name="const", bufs=1))
        persist = ctx.enter_context(tc.tile_pool(name="persist", bufs=1))
        xio = ctx.enter_context(tc.tile_pool(name="xio", bufs=3))
        vxp = ctx.enter_context(tc.tile_pool(name="vxp", bufs=2))
        strip_pool = ctx.enter_context(tc.tile_pool(name="strip", bufs=4))
        rawp = ctx.enter_context(tc.tile_pool(name="rawp", bufs=2))
        sm = ctx.enter_context(tc.tile_pool(name="sm", bufs=2))
        scps = ctx.enter_context(tc.tile_pool(name="scps", bufs=2, space="PSUM"))
        avps = ctx.enter_context(tc.tile_pool(name="avps", bufs=1, space="PSUM"))
        pjps = ctx.enter_context(tc.tile_pool(name="pjps", bufs=2, space="PSUM"))

        # ---- weights / biases ----
        wq_sb = const.tile([128, KD, CW], BF16, tag="wq")
        wk_sb = const.tile([128, KD, CW], BF16, tag="wk")
        wv_sb = const.tile([128, KD, CW], BF16, tag="wv")
        wo_sb = const.tile([128, CW // 128, D], BF16, tag="wo")
        # strict need-order: proj_k(0) first, then proj_q(0,0), then proj_v
        nc.sync.dma_start(out=wk_sb, in_=wk_d.rearrange("(k p) c -> p k c", p=128))

        # ---- persistent activations ----
        qT_sb = [persist.tile([128, S], BF16, tag=f"qT{p}", name=f"qT_sb{p}") for p in range(2)]
        kT_sb = [persist.tile([128, S], BF16, tag=f"kT{p}", name=f"kT_sb{p}") for p in range(2)]
        v_sb = persist.tile([128, NKT, HPC, DH + 1], BF16, tag="v")
        nc.vector.memset(v_sb[:, :, :, DH : DH + 1], 1.0)
        ao_sb = persist.tile([128, CW // 128, S], BF16, tag="ao")
        osb = persist.tile([128, KD, SBLK], F32, tag="osb")
        # softmax row-sums, one [2, SBLK] slot per (pair, blk)
        sums = [persist.tile([2, SBLK], F32, tag=f"sums{p}{b}", name=f"sums{p}{b}")
                for p in range(2) for b in range(NSBLK)]

        qTr = qT_d.rearrange("(k p) s -> p k s", p=128)
        kTr = kTx_d.rearrange("(k p) s -> p k s", p=128)
        vTr = vTx_d.rearrange("(k p) s -> p k s", p=128)

        # resident vx halves (loaded once, used by all proj_v tiles)
        vx_half = [None, None]

        def load_vx(half):
            vx = vxp.tile([128, KD, S // 2], BF16, tag="vx", name=f"vx{half}")
            nc.sync.dma_start(out=vx, in_=vTr[:, :, half * (S // 2) : (half + 1) * (S // 2)])
            vx_half[half] = vx

        xk0 = xio.tile([128, KD, SBLK], BF16, tag="x", name="xk0pre")
        nc.sync.dma_start(out=xk0, in_=kTr[:, :, 0:SBLK])
        bk_sb = const.tile([128, 2], F32, tag="bk")
        bq_sb = const.tile([128, 2], F32, tag="bq")
        for p in range(2):
            nc.sync.dma_start(out=bk_sb[:, p : p + 1],
                              in_=bk_d[p * 128 : (p + 1) * 128].unsqueeze(1))
        nc.sync.dma_start(out=wq_sb, in_=wq_d.rearrange("(k p) c -> p k c", p=128))
        xq00 = xio.tile([128, KD, SBLK], BF16, tag="x", name="xq00pre")
        nc.sync.dma_start(out=xq00, in_=qTr[:, :, 0:SBLK])
        for p in range(2):
            nc.sync.dma_start(out=bq_sb[:, p : p + 1],
                              in_=bq_d[p * 128 : (p + 1) * 128].unsqueeze(1))
        nc.sync.dma_start(out=wv_sb, in_=wv_d.rearrange("(k p) c -> p k c", p=128))
        load_vx(0)
        bv_row = const.tile([1, CW], F32, tag="bvr")
        nc.sync.dma_start(out=bv_row, in_=bv_d.unsqueeze(0))
        bv_bc = const.tile([128, CW], F32, tag="bvb")
        nc.gpsimd.partition_broadcast(bv_bc, bv_row)
        nc.sync.dma_start(out=wo_sb, in_=wo_d.rearrange("(ct p) e -> p ct e", p=128))
        pre_x = {("k", 0, 0): xk0, ("k", 1, 0): xk0, ("q", 0, 0): xq00}

        # ---------------- projection helpers ----------------
        proj_state = {}

        NCHUNK = 4  # contraction chunks per q/k projection (2 matmuls each)

        def proj_qk(which, p, sb, chunk):
            """Quarter of a q/k projection (2 of 8 contraction tiles) so
            filler pops stay fine-grained (~0.5us each). chunk 0 loads x and
            starts the psum; the last chunk moves psum->SBUF with bias add."""
            w_sb, b_sb, dst, xr = (
                (wq_sb, bq_sb, qT_sb, qTr) if which == "q" else (wk_sb, bk_sb, kT_sb, kTr))
            key = (which, p, sb)
            if chunk == 0:
                x = pre_x.get(key)
                if x is None:
                    x = xio.tile([128, KD, SBLK], BF16, tag="x", name=f"x{which}{p}{sb}")
                    nc.sync.dma_start(out=x, in_=xr[:, :, sb * SBLK : (sb + 1) * SBLK])
                ps = pjps.tile([128, SBLK], F32, tag="pj", name=f"ps{which}{p}{sb}")
                proj_state[key] = (x, ps)
            x, ps = proj_state[key]
            for k in range(chunk * KD // NCHUNK, (chunk + 1) * KD // NCHUNK):
                nc.tensor.matmul(ps, w_sb[:, k, p * 128 : (p + 1) * 128],
                                 x[:, k, :], start=(k == 0), stop=(k == KD - 1))
            if chunk == NCHUNK - 1:
                nc.vector.tensor_scalar_add(dst[p][:, sb * SBLK : (sb + 1) * SBLK],
                                            ps, b_sb[:, p : p + 1])
                del proj_state[key]

        def proj_k(sb, pairs=(0, 1)):
            for p in pairs:
                for c in range(NCHUNK):
                    proj_qk("k", p, sb, c)

        def proj_q(p, sb):
            for c in range(NCHUNK):
                proj_qk("q", p, sb, c)

        def proj_v_tile(t):
            """v (natural layout) for s_k tile t, all 4 heads at once."""
            half, ti = divmod(t, NKT // 2)
            vx = vx_half[half]
            psv = pjps.tile([128, CW], F32, tag="pj", name=f"psv{t}")
            for k in range(KD):
                nc.tensor.matmul(psv, vx[:, k, ti * 128 : (ti + 1) * 128],
                                 wv_sb[:, k, :], start=(k == 0), stop=(k == KD - 1))
            nc.vector.tensor_add(
                v_sb[:, t, :, 0:DH],
                psv.rearrange("p (h c) -> p h c", h=HPC),
                bv_bc.rearrange("p (h c) -> p h c", h=HPC),
            )

        # ---------------- attention ----------------
        def attn(p, blk, filler):
            """Attention for (pair, query-block). `filler` is a list of
            zero-arg closures; a few are popped between groups to keep the
            PE busy while the Activation engine catches up on exp."""
            sq = slice(blk * SBLK, (blk + 1) * SBLK)
            avA = avps.tile([DH + 1, SBLK], F32, tag="avA", name=f"avA{p}{blk}")
            avB = avps.tile([DH + 1, SBLK], F32, tag="avB", name=f"avB{p}{blk}")
            strips = {}
            def emit_av(g):
                # AV matmuls for group g (lagged one group behind scores so
                # the PE never blocks on the Activation engine's exp)
                half = (g * SC_G) // (NKT // 2)
                expA, expB = strips[(0, half)], strips[(1, half)]
                for i in range(SC_G):
                    t = g * SC_G + i
                    th = t % (NKT // 2)
                    nc.tensor.matmul(avA, v_sb[:, t, 2 * p, :], expA[:, th, :],
                                     start=(t == 0), stop=(t == NKT - 1))
                for i in range(SC_G):
                    t = g * SC_G + i
                    th = t % (NKT // 2)
                    nc.tensor.matmul(avB, v_sb[:, t, 2 * p + 1, :], expB[:, th, :],
                                     start=(t == 0), stop=(t == NKT - 1))

            for g in range(NGRP):
                t0 = g * SC_G
                half, gh = divmod(g, NGRP // 2)
                if gh == 0:
                    strips[(0, half)] = strip_pool.tile(
                        [128, NKT // 2, SBLK], BF16, tag="exp", name=f"expA{p}{blk}{half}")
                    strips[(1, half)] = strip_pool.tile(
                        [128, NKT // 2, SBLK], BF16, tag="exp", name=f"expB{p}{blk}{half}")
                expA, expB = strips[(0, half)], strips[(1, half)]
                th0 = gh * SC_G
                scA = scps.tile([128, SC_G, SBLK], F32, tag="sc", name=f"scA{p}{blk}{g}")
                for i in range(SC_G):
                    t = t0 + i
                    nc.tensor.matmul(scA[:, i, :],
                                     kT_sb[p][0:64, t * 128 : (t + 1) * 128],
                                     qT_sb[p][0:64, sq], start=True, stop=True)
                nc.scalar.activation(expA[:, th0 : th0 + SC_G, :], scA, AF.Exp, scale=SCALE)
                scB = scps.tile([128, SC_G, SBLK], F32, tag="sc", name=f"scB{p}{blk}{g}")
                for i in range(SC_G):
                    t = t0 + i
                    nc.tensor.matmul(scB[:, i, :],
                                     kT_sb[p][64:128, t * 128 : (t + 1) * 128],
                                     qT_sb[p][64:128, sq], start=True, stop=True)
                nc.scalar.activation(expB[:, th0 : th0 + SC_G, :], scB, AF.Exp, scale=SCALE)
                # PE filler while the Activation engine computes this group's exp
                for fn in filler.pop_for_group(p, blk, g):
                    fn()
                if g > 0:
                    emit_av(g - 1)
            emit_av(NGRP - 1)
            return avA, avB

        # per (pair, blk): drain av psum to SBUF, extract row sums
        raw_tiles = {}

        def drain_av(p, blk, avA, avB):
            rawA = rawp.tile([DH + 1, SBLK], F32, tag="rawA", name=f"rawA{p}{blk}")
            rawB = rawp.tile([DH + 1, SBLK], F32, tag="rawB", name=f"rawB{p}{blk}")
            nc.vector.tensor_copy(rawA, avA)
            nc.vector.tensor_copy(rawB, avB)
            s = sums[p * NSBLK + blk]
            nc.sync.dma_start(out=s[0:1, :], in_=rawA[DH : DH + 1, :])
            nc.sync.dma_start(out=s[1:2, :], in_=rawB[DH : DH + 1, :])
            nc.vector.reciprocal_approx_fast(out=s, in_=s)
            raw_tiles[(p, blk)] = (rawA, rawB)

        def normalize(p, blk):
            sq = slice(blk * SBLK, (blk + 1) * SBLK)
            rawA, rawB = raw_tiles.pop((p, blk))
            s = sums[p * NSBLK + blk]
            for j, raw in enumerate((rawA, rawB)):
                rcp0 = sm.tile([1, SBLK], F32, tag="rcp0", name=f"rcp0{p}{blk}{j}")
                nc.sync.dma_start(out=rcp0, in_=s[j : j + 1, :])
                rcpb = sm.tile([64, SBLK], F32, tag="rcpb", name=f"rcpb{p}{blk}{j}")
                nc.gpsimd.partition_broadcast(rcpb, rcp0)
                if j == 0:
                    nc.vector.tensor_mul(ao_sb[0:64, p, sq], raw[0:DH, :], rcpb)
                else:
                    stage = sm.tile([64, SBLK], BF16, tag="stage", name=f"stage{p}{blk}{j}")
                    nc.vector.tensor_mul(stage, raw[0:DH, :], rcpb)
                    nc.sync.dma_start(out=ao_sb[64:128, p, sq], in_=stage)

        def outproj_et(blk, et, dma_now=False):
            sq = slice(blk * SBLK, (blk + 1) * SBLK)
            pso = pjps.tile([128, SBLK], F32, tag="pj", name=f"pso{blk}{et}")
            for ct in range(CW // 128):
                nc.tensor.matmul(pso, wo_sb[:, ct, et * 128 : (et + 1) * 128],
                                 ao_sb[:, ct, sq], start=(ct == 0),
                                 stop=(ct == CW // 128 - 1))
            if dma_now and et % 2 == 1:
                nc.scalar.copy(osb[:, et, :], pso)
            else:
                nc.vector.tensor_copy(osb[:, et, :], pso)
            if dma_now:
                nc.sync.dma_start(
                    out=out_d.rearrange("(et p) s -> p et s", p=128)[:, et, sq],
                    in_=osb[:, et, :])

        def out_dma(blk):
            sq = slice(blk * SBLK, (blk + 1) * SBLK)
            nc.sync.dma_start(
                out=out_d.rearrange("(et p) s -> p et s", p=128)[:, :, sq],
                in_=osb)

        # ---------------- filler scheduling ----------------
        class Filler:
            """Pinned (JIT) emissions + a generic weighted queue popped
            between groups (weight ~ matmul count, budget ~3 per group)."""

            def __init__(self):
                self.queue = []  # list of (weight, fn)

            def add(self, weight, fn):
                self.queue.append((weight, fn))

            def pop_for_group(self, p, blk, g):
                out = []
                if p == 0 and blk == 0:
                    # JIT: k-projection for upcoming s-blocks, v tiles for
                    # this group, q-projection for the next block
                    if g % 2 == 1 and g < NGRP - 1:
                        out.append(lambda sb=(g + 1) // 2: proj_k(sb))
                    if g == 1:
                        out.append(lambda: load_vx(1))
                    if g == NGRP - 2:
                        out.append(lambda: proj_q(0, 1))
                    if g >= 1:
                        for t in range((g - 1) * SC_G, g * SC_G):
                            out.append(lambda t=t: proj_v_tile(t))
                        if g == NGRP - 1:
                            for t in range(g * SC_G, (g + 1) * SC_G):
                                out.append(lambda t=t: proj_v_tile(t))
                else:
                    w = 0
                    while self.queue and w < 3:
                        wi, fn = self.queue.pop(0)
                        w += wi
                        out.append(fn)
                return out

        filler = Filler()

        # ---------------- emission schedule ----------------
        proj_k(0)
        proj_q(0, 0)

        # pair 0: block 0 streams k/v projections just-in-time via Filler
        avA, avB = attn(0, 0, filler)
        drain_av(0, 0, avA, avB)

        # generic filler for pair-0 blocks 1..3: remaining q projections in
        # half-projection chunks (proj_q(0, 1) was pinned inside block 0)
        for p, sb in [(0, 2), (0, 3), (1, 0), (1, 1), (1, 2), (1, 3)]:
            filler.add(8, lambda p=p, sb=sb: proj_q(p, sb))

        for blk in range(1, NSBLK):
            avA, avB = attn(0, blk, filler)
            drain_av(0, blk, avA, avB)
            normalize(0, blk - 1)

        # pair 1 blocks; outproj(blk-1) interleaves as PE filler
        normalize(0, NSBLK - 1)
        for blk in range(NSBLK):
            if blk > 0:
                for et in range(KD):
                    filler.add(2, lambda b=blk - 1, et=et: outproj_et(b, et))
                filler.add(0, lambda b=blk - 1: out_dma(b))
            avA, avB = attn(1, blk, filler)
            drain_av(1, blk, avA, avB)
            normalize(1, blk)
        # drain remaining filler (if any), then the final block's outproj
        for _, fn in filler.queue:
            fn()
        filler.queue = []
        for et in range(KD):
            outproj_et(NSBLK - 1, et, dma_now=True)


def _get_program():
    if "nc" not in _CACHE:
        _CACHE["nc"] = _build_program()
    return _CACHE["nc"]


LAST_RESULTS = None


def kernel(query, key_, value, Wq, bq, Wk, bk, Wv, bv, Wo, bo):
    global LAST_RESULTS
    query = np.asarray(query, dtype=np.float32)
    key_ = np.asarray(key_, dtype=np.float32)
    value = np.asarray(value, dtype=np.float32)
    Wq = np.asarray(Wq, dtype=np.float32)
    Wk = np.asarray(Wk, dtype=np.float32)
    Wv = np.asarray(Wv, dtype=np.float32)
    Wo = np.asarray(Wo, dtype=np.float32)
    bq = np.asarray(bq, dtype=np.float32)
    bk = np.asarray(bk, dtype=np.float32)
    bv = np.asarray(bv, dtype=np.float32)
    bo = np.asarray(bo, dtype=np.float32)

    nc = _get_program()

    BF = ml_dtypes.bfloat16
    qT = [np.ascontiguousarray(query[b].T.astype(BF)) for b in range(B)]
    kT = [np.ascontiguousarray(key_[b].T.astype(BF)) for b in range(B)]
    vT = [np.ascontiguousarray(value[b].T.astype(BF)) for b in range(B)]

    in_maps = []
    for c in range(N_CORES):
        b, hp = divmod(c, HPC)
        cs = slice(hp * CW, (hp + 1) * CW)
        in_maps.append({
            "qT": qT[b], "kTx": kT[b], "vTx": vT[b],
            "wqT": np.ascontiguousarray(Wq[cs, :].T.astype(BF)),
            "wkT": np.ascontiguousarray(Wk[cs, :].T.astype(BF)),
            "wvT": np.ascontiguousarray(Wv[cs, :].T.astype(BF)),
            "woT": np.ascontiguousarray(Wo[:, cs].T.astype(BF)),
            "bq": np.ascontiguousarray(bq[cs]),
            "bk": np.ascontiguousarray(bk[cs]),
            "bv": np.ascontiguousarray(bv[cs]),
        })

    res = bass_utils.run_bass_kernel_spmd(nc, in_maps, core_ids=list(range(N_CORES)))
    LAST_RESULTS = res

    out = np.zeros((B, S, D), dtype=np.float32)
    for c in range(N_CORES):
        b = c // HPC
        out[b] += res.results[c]["outT"].T
    out += bo
    return out


# revision 46
# speedup vs baseline: 1.0380x; 1.0377x over previous
"""Multi-head attention Trainium2 kernel (B=2, S=2048, D=1024, H=16, Dh=64).

Sharding: 8 cores = 2 (batch) x 4 (head-groups of 4 heads).
Each core computes qT/kT/v projections for its 4 heads, attention
(scoresT layout, fused softmax-sum via an extra ones-column in V),
and a partial (row-sharded) output projection. Host sums the 4 head-group
partials per batch and adds bo.

Schedule: fine-grained software pipeline. Attention for pair 0 / block 0
starts after a ~24-matmul prologue; the remaining k/v projections stream
just-in-time inside block 0, and q/output projections are interleaved as
PE filler between score/AV groups so the tensor engine never stalls on
the Activation engine's exp. Softmax normalization is staged through
SBUF with DMA partition shifts and per-block batched reciprocals; the
output-projection PSUM drains through the (otherwise idle) Pool engine.
"""

import sys

sys.path.insert(0, "/opt/trn_rl_repo")

import ml_dtypes
import numpy as np

import concourse.bass as bass  # noqa: F401
import concourse.mybir as mybir
import concourse.tile as tile
from concourse import bacc, bass_utils

F32 = mybir.dt.float32
BF16 = mybir.dt.bfloat16
AF = mybir.ActivationFunctionType

B, S, D = 2, 2048, 1024
H, DH = 16, 64
N_CORES = 8
HPC = 4  # heads per core
CW = HPC * DH  # c-width per core (256)
SBLK = 512  # s_q block size
NSBLK = S // SBLK  # 4
NKT = S // 128  # 16 s_k tiles
KD = D // 128  # 8 contraction tiles for projections
SC_G = 2  # s_k tiles per scores psum group
NGRP = NKT // SC_G  # 8 groups per (pair, block)

_CACHE = {}


def _build_program():
    nc = bacc.Bacc("TRN2", target_bir_lowering=False, debug=False, num_devices=N_CORES)

    qT_d = nc.dram_tensor("qT", [D, S], BF16, kind="ExternalInput").ap()
    kTx_d = nc.dram_tensor("kTx", [D, S], BF16, kind="ExternalInput").ap()
    vTx_d = nc.dram_tensor("vTx", [D, S], BF16, kind="ExternalInput").ap()
    wq_d = nc.dram_tensor("wqT", [D, CW], BF16, kind="ExternalInput").ap()
    wk_d = nc.dram_tensor("wkT", [D, CW], BF16, kind="ExternalInput").ap()
    wv_d = nc.dram_tensor("wvT", [D, CW], BF16, kind="ExternalInput").ap()
    wo_d = nc.dram_tensor("woT", [CW, D], BF16, kind="ExternalInput").ap()
    bq_d = nc.dram_tensor("bq", [CW], F32, kind="ExternalInput").ap()
    bk_d = nc.dram_tensor("bk", [CW], F32, kind="ExternalInput").ap()
    bv_d = nc.dram_tensor("bv", [CW], F32, kind="ExternalInput").ap()
    out_d = nc.dram_tensor("outT", [D, S], F32, kind="ExternalOutput").ap()

    with tile.TileContext(nc) as tc:
        _kernel_body(nc, tc, qT_d, kTx_d, vTx_d, wq_d, wk_d, wv_d, wo_d,
                     bq_d, bk_d, bv_d, out_d)
    nc.compile()
    return nc


def _kernel_body(nc, tc, qT_d, kTx_d, vTx_d, wq_d, wk_d, wv_d, wo_d,
                 bq_d, bk_d, bv_d, out_d):
    from contextlib import ExitStack

    SCALE = float(1.0 / np.sqrt(DH))
    # Schraudolph-style bf16 exp on the DVE: exp(s*SCALE) ~= bitcast_bf16(
    # int16(round(s * SCALE * 2^7/ln2 + (127*2^7 - 4.7 - 2.8)))).
    # The +1.5% mean bias of the approximation cancels in softmax since the
    # row-sums are computed from the same approximated probabilities.
    EXP_A = SCALE * 184.6650085
    EXP_B = 16248.5

    ctx = ExitStack()
    with ctx:
        const = ctx.enter_context(tc.tile_pool(name="const", bufs=1))
        persist = ctx.enter_context(tc.tile_pool(name="persist", bufs=1))
        xio = ctx.enter_context(tc.tile_pool(name="xio", bufs=3))
        vxp = ctx.enter_context(tc.tile_pool(name="vxp", bufs=2))
        strip_pool = ctx.enter_context(tc.tile_pool(name="strip", bufs=4))
        rawp = ctx.enter_context(tc.tile_pool(name="rawp", bufs=2))
        sm = ctx.enter_context(tc.tile_pool(name="sm", bufs=2))
        scps = ctx.enter_context(tc.tile_pool(name="scps", bufs=2, space="PSUM"))
        avps = ctx.enter_context(tc.tile_pool(name="avps", bufs=1, space="PSUM"))
        pjps = ctx.enter_context(tc.tile_pool(name="pjps", bufs=2, space="PSUM"))

        # ---- weights / biases ----
        wq_sb = const.tile([128, KD, CW], BF16, tag="wq")
        wk_sb = const.tile([128, KD, CW], BF16, tag="wk")
        wv_sb = const.tile([128, KD, CW], BF16, tag="wv")
        wo_sb = const.tile([128, CW // 128, D], BF16, tag="wo")
        # strict need-order: proj_k(0) first, then proj_q(0,0), then proj_v
        nc.sync.dma_start(out=wk_sb, in_=wk_d.rearrange("(k p) c -> p k c", p=128))

        # ---- persistent activations ----
        qT_sb = [persist.tile([128, S], BF16, tag=f"qT{p}", name=f"qT_sb{p}") for p in range(2)]
        kT_sb = [persist.tile([128, S], BF16, tag=f"kT{p}", name=f"kT_sb{p}") for p in range(2)]
        v_sb = persist.tile([128, NKT, HPC, DH + 1], BF16, tag="v")
        nc.vector.memset(v_sb[:, :, :, DH : DH + 1], 1.0)
        ao_sb = persist.tile([128, CW // 128, S], BF16, tag="ao")
        osb = persist.tile([128, KD, SBLK], F32, tag="osb")
        # softmax row-sums, one [2, SBLK] slot per (pair, blk)
        sums = [persist.tile([2, SBLK], F32, tag=f"sums{p}{b}", name=f"sums{p}{b}")
                for p in range(2) for b in range(NSBLK)]

        qTr = qT_d.rearrange("(k p) s -> p k s", p=128)
        kTr = kTx_d.rearrange("(k p) s -> p k s", p=128)
        vTr = vTx_d.rearrange("(k p) s -> p k s", p=128)

        # resident vx halves (loaded once, used by all proj_v tiles)
        vx_half = [None, None]

        def load_vx(half):
            vx = vxp.tile([128, KD, S // 2], BF16, tag="vx", name=f"vx{half}")
            nc.sync.dma_start(out=vx, in_=vTr[:, :, half * (S // 2) : (half + 1) * (S // 2)])
            vx_half[half] = vx

        xk0 = xio.tile([128, KD, SBLK], BF16, tag="x", name="xk0pre")
        nc.sync.dma_start(out=xk0, in_=kTr[:, :, 0:SBLK])
        bk_sb = const.tile([128, 2], F32, tag="bk")
        bq_sb = const.tile([128, 2], F32, tag="bq")
        for p in range(2):
            nc.sync.dma_start(out=bk_sb[:, p : p + 1],
                              in_=bk_d[p * 128 : (p + 1) * 128].unsqueeze(1))
        nc.sync.dma_start(out=wq_sb, in_=wq_d.rearrange("(k p) c -> p k c", p=128))
        xq00 = xio.tile([128, KD, SBLK], BF16, tag="x", name="xq00pre")
        nc.sync.dma_start(out=xq00, in_=qTr[:, :, 0:SBLK])
        for p in range(2):
            nc.sync.dma_start(out=bq_sb[:, p : p + 1],
                              in_=bq_d[p * 128 : (p + 1) * 128].unsqueeze(1))
        nc.sync.dma_start(out=wv_sb, in_=wv_d.rearrange("(k p) c -> p k c", p=128))
        load_vx(0)
        bv_row = const.tile([1, CW], F32, tag="bvr")
        nc.sync.dma_start(out=bv_row, in_=bv_d.unsqueeze(0))
        bv_bc = const.tile([128, CW], F32, tag="bvb")
        nc.gpsimd.partition_broadcast(bv_bc, bv_row)
        nc.sync.dma_start(out=wo_sb, in_=wo_d.rearrange("(ct p) e -> p ct e", p=128))
        pre_x = {("k", 0, 0): xk0, ("k", 1, 0): xk0, ("q", 0, 0): xq00}

        # ---------------- projection helpers ----------------
        proj_state = {}

        NCHUNK = 4  # contraction chunks per q/k projection (2 matmuls each)

        def proj_qk(which, p, sb, chunk):
            """Quarter of a q/k projection (2 of 8 contraction tiles) so
            filler pops stay fine-grained (~0.5us each). chunk 0 loads x and
            starts the psum; the last chunk moves psum->SBUF with bias add."""
            w_sb, b_sb, dst, xr = (
                (wq_sb, bq_sb, qT_sb, qTr) if which == "q" else (wk_sb, bk_sb, kT_sb, kTr))
            key = (which, p, sb)
            if chunk == 0:
                x = pre_x.get(key)
                if x is None:
                    x = xio.tile([128, KD, SBLK], BF16, tag="x", name=f"x{which}{p}{sb}")
                    nc.sync.dma_start(out=x, in_=xr[:, :, sb * SBLK : (sb + 1) * SBLK])
                ps = pjps.tile([128, SBLK], F32, tag="pj", name=f"ps{which}{p}{sb}")
                proj_state[key] = (x, ps)
            x, ps = proj_state[key]
            for k in range(chunk * KD // NCHUNK, (chunk + 1) * KD // NCHUNK):
                nc.tensor.matmul(ps, w_sb[:, k, p * 128 : (p + 1) * 128],
                                 x[:, k, :], start=(k == 0), stop=(k == KD - 1))
            if chunk == NCHUNK - 1:
                nc.vector.tensor_scalar_add(dst[p][:, sb * SBLK : (sb + 1) * SBLK],
                                            ps, b_sb[:, p : p + 1])
                del proj_state[key]

        def proj_k(sb, pairs=(0, 1)):
            for p in pairs:
                for c in range(NCHUNK):
                    proj_qk("k", p, sb, c)

        def proj_q(p, sb):
            for c in range(NCHUNK):
                proj_qk("q", p, sb, c)

        def proj_v_tile(t):
            """v (natural layout) for s_k tile t, all 4 heads at once."""
            half, ti = divmod(t, NKT // 2)
            vx = vx_half[half]
            psv = pjps.tile([128, CW], F32, tag="pj", name=f"psv{t}")
            for k in range(KD):
                nc.tensor.matmul(psv, vx[:, k, ti * 128 : (ti + 1) * 128],
                                 wv_sb[:, k, :], start=(k == 0), stop=(k == KD - 1))
            nc.vector.tensor_add(
                v_sb[:, t, :, 0:DH],
                psv.rearrange("p (h c) -> p h c", h=HPC),
                bv_bc.rearrange("p (h c) -> p h c", h=HPC),
            )

        # ---------------- attention ----------------
        def attn(p, blk, filler):
            """Attention for (pair, query-block). `filler` is a list of
            zero-arg closures; a few are popped between groups to keep the
            PE busy while the Activation engine catches up on exp."""
            sq = slice(blk * SBLK, (blk + 1) * SBLK)
            avA = avps.tile([DH + 1, SBLK], F32, tag="avA", name=f"avA{p}{blk}")
            avB = avps.tile([DH + 1, SBLK], F32, tag="avB", name=f"avB{p}{blk}")
            strips = {}
            def emit_av(g):
                # AV matmuls for group g (lagged one group behind scores so
                # the PE never blocks on the Activation engine's exp)
                half = (g * SC_G) // (NKT // 2)
                expA, expB = strips[(0, half)], strips[(1, half)]
                for i in range(SC_G):
                    t = g * SC_G + i
                    th = t % (NKT // 2)
                    nc.tensor.matmul(avA, v_sb[:, t, 2 * p, :], expA[:, th, :],
                                     start=(t == 0), stop=(t == NKT - 1))
                for i in range(SC_G):
                    t = g * SC_G + i
                    th = t % (NKT // 2)
                    nc.tensor.matmul(avB, v_sb[:, t, 2 * p + 1, :], expB[:, th, :],
                                     start=(t == 0), stop=(t == NKT - 1))

            for g in range(NGRP):
                t0 = g * SC_G
                half, gh = divmod(g, NGRP // 2)
                if gh == 0:
                    strips[(0, half)] = strip_pool.tile(
                        [128, NKT // 2, SBLK], BF16, tag="exp", name=f"expA{p}{blk}{half}")
                    strips[(1, half)] = strip_pool.tile(
                        [128, NKT // 2, SBLK], BF16, tag="exp", name=f"expB{p}{blk}{half}")
                expA, expB = strips[(0, half)], strips[(1, half)]
                th0 = gh * SC_G
                scA = scps.tile([128, SC_G, SBLK], F32, tag="sc", name=f"scA{p}{blk}{g}")
                for i in range(SC_G):
                    t = t0 + i
                    nc.tensor.matmul(scA[:, i, :],
                                     kT_sb[p][0:64, t * 128 : (t + 1) * 128],
                                     qT_sb[p][0:64, sq], start=True, stop=True)
                nc.scalar.activation(expA[:, th0 : th0 + SC_G, :], scA, AF.Exp, scale=SCALE)
                scB = scps.tile([128, SC_G, SBLK], F32, tag="sc", name=f"scB{p}{blk}{g}")
                for i in range(SC_G):
                    t = t0 + i
                    nc.tensor.matmul(scB[:, i, :],
                                     kT_sb[p][64:128, t * 128 : (t + 1) * 128],
                                     qT_sb[p][64:128, sq], start=True, stop=True)
                nc.scalar.activation(expB[:, th0 : th0 + SC_G, :], scB, AF.Exp, scale=SCALE)
                # PE filler while the Activation engine computes this group's exp
                for fn in filler.pop_for_group(p, blk, g):
                    fn()
                if g > 0:
                    emit_av(g - 1)
            emit_av(NGRP - 1)
            return avA, avB

        # per (pair, blk): drain av psum to SBUF, extract row sums
        raw_tiles = {}

        def drain_av(p, blk, avA, avB):
            rawA = rawp.tile([DH + 1, SBLK], F32, tag="rawA", name=f"rawA{p}{blk}")
            rawB = rawp.tile([DH + 1, SBLK], F32, tag="rawB", name=f"rawB{p}{blk}")
            nc.vector.tensor_copy(rawA, avA)
            nc.vector.tensor_copy(rawB, avB)
            s = sums[p * NSBLK + blk]
            nc.sync.dma_start(out=s[0:1, :], in_=rawA[DH : DH + 1, :])
            nc.sync.dma_start(out=s[1:2, :], in_=rawB[DH : DH + 1, :])
            nc.vector.reciprocal_approx_fast(out=s, in_=s)
            raw_tiles[(p, blk)] = (rawA, rawB)

        def normalize(p, blk):
            sq = slice(blk * SBLK, (blk + 1) * SBLK)
            rawA, rawB = raw_tiles.pop((p, blk))
            s = sums[p * NSBLK + blk]
            for j, raw in enumerate((rawA, rawB)):
                rcp0 = sm.tile([1, SBLK], F32, tag="rcp0", name=f"rcp0{p}{blk}{j}")
                nc.sync.dma_start(out=rcp0, in_=s[j : j + 1, :])
                rcpb = sm.tile([64, SBLK], F32, tag="rcpb", name=f"rcpb{p}{blk}{j}")
                nc.gpsimd.partition_broadcast(rcpb, rcp0)
                if j == 0:
                    nc.vector.tensor_mul(ao_sb[0:64, p, sq], raw[0:DH, :], rcpb)
                else:
                    stage = sm.tile([64, SBLK], BF16, tag="stage", name=f"stage{p}{blk}{j}")
                    nc.vector.tensor_mul(stage, raw[0:DH, :], rcpb)
                    nc.sync.dma_start(out=ao_sb[64:128, p, sq], in_=stage)

        def outproj_et(blk, et, dma_now=False):
            sq = slice(blk * SBLK, (blk + 1) * SBLK)
            pso = pjps.tile([128, SBLK], F32, tag="pj", name=f"pso{blk}{et}")
            for ct in range(CW // 128):
                nc.tensor.matmul(pso, wo_sb[:, ct, et * 128 : (et + 1) * 128],
                                 ao_sb[:, ct, sq], start=(ct == 0),
                                 stop=(ct == CW // 128 - 1))
            nc.vector.tensor_copy(osb[:, et, :], pso)
            if dma_now:
                nc.sync.dma_start(
                    out=out_d.rearrange("(et p) s -> p et s", p=128)[:, et, sq],
                    in_=osb[:, et, :])

        def out_dma(blk):
            sq = slice(blk * SBLK, (blk + 1) * SBLK)
            nc.sync.dma_start(
                out=out_d.rearrange("(et p) s -> p et s", p=128)[:, :, sq],
                in_=osb)

        # ---------------- filler scheduling ----------------
        class Filler:
            """Pinned (JIT) emissions + a generic weighted queue popped
            between groups (weight ~ matmul count, budget ~3 per group)."""

            def __init__(self):
                self.queue = []  # list of (weight, fn)

            def add(self, weight, fn):
                self.queue.append((weight, fn))

            def pop_for_group(self, p, blk, g):
                out = []
                if p == 0 and blk == 0:
                    # JIT: k-projection for upcoming s-blocks, v tiles for
                    # this group, q-projection for the next block
                    if g % 2 == 1 and g < NGRP - 1:
                        out.append(lambda sb=(g + 1) // 2: proj_k(sb))
                    if g == 1:
                        out.append(lambda: load_vx(1))
                    if g == NGRP - 2:
                        out.append(lambda: proj_q(0, 1))
                    for t in range(g * SC_G, (g + 1) * SC_G):
                        out.append(lambda t=t: proj_v_tile(t))
                else:
                    w = 0
                    while self.queue and w < 3:
                        wi, fn = self.queue.pop(0)
                        w += wi
                        out.append(fn)
                return out

        filler = Filler()

        # ---------------- emission schedule ----------------
        proj_k(0)
        proj_q(0, 0)

        # pair 0: block 0 streams k/v projections just-in-time via Filler
        avA, avB = attn(0, 0, filler)
        drain_av(0, 0, avA, avB)

        # generic filler for pair-0 blocks 1..3: remaining q projections in
        # half-projection chunks (proj_q(0, 1) was pinned inside block 0)
        for p, sb in [(0, 2), (0, 3), (1, 0), (1, 1), (1, 2), (1, 3)]:
            filler.add(8, lambda p=p, sb=sb: proj_q(p, sb))

        for blk in range(1, NSBLK):
            avA, avB = attn(0, blk, filler)
            drain_av(0, blk, avA, avB)
            normalize(0, blk - 1)

        # pair 1 blocks; outproj(blk-1) interleaves as PE filler
        normalize(0, NSBLK - 1)
        for blk in range(NSBLK):
            if blk > 0:
                for et in range(KD):
                    filler.add(2, lambda b=blk - 1, et=et: outproj_et(b, et))
                filler.add(0, lambda b=blk - 1: out_dma(b))
            avA, avB = attn(1, blk, filler)
            drain_av(1, blk, avA, avB)
            normalize(1, blk)
        # drain remaining filler (if any), then the final block's outproj
        for _, fn in filler.queue:
            fn()
        filler.queue = []
        for et in range(KD):
            outproj_et(NSBLK - 1, et, dma_now=True)


def _get_program():
    if "nc" not in _CACHE:
        _CACHE["nc"] = _build_program()
    return _CACHE["nc"]


LAST_RESULTS = None


def kernel(query, key_, value, Wq, bq, Wk, bk, Wv, bv, Wo, bo):
    global LAST_RESULTS
    query = np.asarray(query, dtype=np.float32)
    key_ = np.asarray(key_, dtype=np.float32)
    value = np.asarray(value, dtype=np.float32)
    Wq = np.asarray(Wq, dtype=np.float32)
    Wk = np.asarray(Wk, dtype=np.float32)
    Wv = np.asarray(Wv, dtype=np.float32)
    Wo = np.asarray(Wo, dtype=np.float32)
    bq = np.asarray(bq, dtype=np.float32)
    bk = np.asarray(bk, dtype=np.float32)
    bv = np.asarray(bv, dtype=np.float32)
    bo = np.asarray(bo, dtype=np.float32)

    nc = _get_program()

    BF = ml_dtypes.bfloat16
    qT = [np.ascontiguousarray(query[b].T.astype(BF)) for b in range(B)]
    kT = [np.ascontiguousarray(key_[b].T.astype(BF)) for b in range(B)]
    vT = [np.ascontiguousarray(value[b].T.astype(BF)) for b in range(B)]

    in_maps = []
    for c in range(N_CORES):
        b, hp = divmod(c, HPC)
        cs = slice(hp * CW, (hp + 1) * CW)
        in_maps.append({
            "qT": qT[b], "kTx": kT[b], "vTx": vT[b],
            "wqT": np.ascontiguousarray(Wq[cs, :].T.astype(BF)),
            "wkT": np.ascontiguousarray(Wk[cs, :].T.astype(BF)),
            "wvT": np.ascontiguousarray(Wv[cs, :].T.astype(BF)),
            "woT": np.ascontiguousarray(Wo[:, cs].T.astype(BF)),
            "bq": np.ascontiguousarray(bq[cs]),
            "bk": np.ascontiguousarray(bk[cs]),
            "bv": np.ascontiguousarray(bv[cs]),
        })

    res = bass_utils.run_bass_kernel_spmd(nc, in_maps, core_ids=list(range(N_CORES)))
    LAST_RESULTS = res

    out = np.zeros((B, S, D), dtype=np.float32)
    for c in range(N_CORES):
        b = c // HPC
        out[b] += res.results[c]["outT"].T
    out += bo
    return out
